# revision 1
# baseline (speedup 1.0000x reference)
"""Trainium2 Bass kernel for nn_CharacterLoss: pairwise-cosine BCE loss.

reference:  x = data[indices]; z = cosine-sim(x, x)  [M, M]
            t = token match;  loss = mean(softplus(z) - z * t)

Math used on-device (this toolchain has no softplus ACT table):
  softplus(z) - z*t = softplus(w),  w = z * (1 - 2t)
  sum_j softplus(w_j) = -ln prod_j sigma(-w_j)
Per [128, 512] tile of z (produced in PSUM by 4 accumulating fp8e4m3
DoubleRow matmuls, contraction D=1024 as 4 k-steps of 256):
  DVE:  sign' = (tok_i == tok_j) - 0.5          (fp16, 2x mode)
  DVE:  v = z * sign'                            (w = -2v)
  ACT:  s = sigmoid(2v)                          (fp16 out)
  DVE:  grouped products of 8 sigmoids -> pacc   (fp16, 2x mode)
and a single tail Ln pass per host-weight class with a fused row-sum
accumulator.  Host negates / weights / sums the partials in float64.
fp8 quantization of the normalized rows is statistically unbiased in the
mean over 16.7M pairs: end-to-end rel err ~1e-7.

Sharding (8 cores, symmetric-half): the pairwise matrix is blocked into
an 8x8 grid of 512x512 blocks.  Core c computes blocks (c, (c+j) mod 8),
j = 0..4 -- 20 [128, 512] tiles.  Host-side weights: diag j=0 -> 1,
j=1..3 -> 2 (covers the transposed block by symmetry), j=4 -> 1 (both
orientations are computed, by core c and core (c+4) mod 8).  All cores
run one identical SPMD program; per-core behavior differs only via the
shipped operands (gather/normalize/transpose/quantize on host is input
prep, per the sharding hint's "row-block of normalized data").

Perf notes (HW-measured via repeat-slope, axon NTFF unavailable):
steady-state ~19-37 us/body (noisy machine), vs 106 us for the first
correct version.  PE is the bottleneck (~21 us PE-only probe; DR
LDWEIGHTS doesn't fully overlap).  GPSIMD tensor_scalar was tried for
the sign op and is catastrophically slow on real HW (~8 us/op) despite
the cost model predicting ~0.85 us -- keep elementwise work off Pool.
"""
import os
import sys

sys.path.insert(0, "/opt/trn_rl_repo")

import numpy as np
import ml_dtypes

import concourse.bass as bass
import concourse.mybir as mybir
import concourse.tile as tile
from concourse import bacc
from concourse.bass_utils import run_bass_kernel_spmd

N_CORES = 8
M = 4096
D = 1024
KT = D // 128  # contraction k-tiles
GROUP = M // N_CORES  # 512 rows per block-group
NBLK = 5  # blocks per core (j = 0..4)
T = 4 * NBLK  # [128, 512] tiles per core
BLOCK_WEIGHTS = np.array([1.0, 2.0, 2.0, 2.0, 1.0])

_cache = {}
last_result = None  # BassKernelResults of the most recent run (for test.py)


def _build(repeat=1, sgn_engine="vector", fp8=True, probe="", pipe=2, sbufs=3, zpbufs=7):
    """fp8=True: operands are fp8e4m3 in DoubleRow layout [p, k', j, col]
    (contraction index d = k'*256 + 2p + j, 4 k-steps of 256); fp8=False:
    bf16 [p, k, col] (8 k-steps of 128)."""
    nc = bacc.Bacc("TRN2", target_bir_lowering=False, debug=False)
    dt = mybir.dt
    if fp8:
        wT_d = nc.dram_tensor(
            "wT", [128, 8 * GROUP], dt.float8e4, kind="ExternalInput"
        ).ap()
        xT_d = nc.dram_tensor(
            "xT", [128, 8 * NBLK * 512], dt.float8e4, kind="ExternalInput"
        ).ap()
    else:
        wT_d = nc.dram_tensor("wT", [D, GROUP], dt.bfloat16, kind="ExternalInput").ap()
        xT_d = nc.dram_tensor(
            "xT", [D, NBLK * 512], dt.bfloat16, kind="ExternalInput"
        ).ap()
    tokx_d = nc.dram_tensor(
        "tokx", [1, NBLK * 512], dt.float16, kind="ExternalInput"
    ).ap()
    tokw_d = nc.dram_tensor("tokw", [128, 4], dt.float32, kind="ExternalInput").ap()
    # two partial sums per repeat: [weight-1 cols, weight-2 cols]
    sp_d = nc.dram_tensor(
        "spacc", [128, 2 * repeat], dt.float32, kind="ExternalOutput"
    ).ap()
    # tiles with x in {0, 4} have host weight 1, x in {1, 2, 3} weight 2
    # product groups of 8: min sigmoid under this data distribution is
    # sigma(-0.17) ~ 0.46 off-diagonal (cos-sims of distinct normalized
    # gaussian rows are < 0.17; exact-duplicate rows are token-matched so
    # s = sigma(z) >= 0.46 too) -> group product >= 0.46^8 = 2e-3, safely
    # normal in fp16, which keeps the DVE reduce in 2-byte fast mode.
    PG = 8
    GC = 512 // PG  # pacc columns per tile
    W1_TILES = [t for t in range(T) if t // 4 in (0, 4)]
    W2_TILES = [t for t in range(T) if t // 4 in (1, 2, 3)]
    pacc_col = {}
    for i, t in enumerate(W1_TILES):
        pacc_col[t] = i * GC
    for i, t in enumerate(W2_TILES):
        pacc_col[t] = len(W1_TILES) * GC + i * GC
    NW1 = len(W1_TILES) * GC

    with tile.TileContext(nc) as tc:
        with (
            tc.tile_pool(name="data", bufs=1) as data_pool,
            tc.tile_pool(name="scratch", bufs=sbufs) as scratch,
            tc.tile_pool(name="ps", bufs=zpbufs, space="PSUM") as ps,
        ):
            # x / w as single SBUF tensors; one mega-DMA per x-block
            # (block-major: tiles are consumed x-major) to avoid per-DMA
            # HWDGE serialization.  w + x-block-0 go first (they gate the
            # first matmuls); tokens follow (needed ~6us in by DVE).
            if fp8:
                wall = data_pool.tile([128, 4, 2, GROUP], dt.float8e4)
                xall = data_pool.tile([128, 4, 2, NBLK * 512], dt.float8e4)
                wT_r = wT_d.rearrange("p (k j c) -> p k j c", k=4, j=2)
                xT_r = xT_d.rearrange("p (k j c) -> p k j c", k=4, j=2)
            else:
                wall = data_pool.tile([128, KT, GROUP], dt.bfloat16)
                xall = data_pool.tile([128, KT, NBLK * 512], dt.bfloat16)
                wT_r = wT_d.rearrange("(k p) c -> p k c", p=128)
                xT_r = xT_d.rearrange("(k p) c -> p k c", p=128)
            tokw = data_pool.tile([128, 4], dt.float32)
            nc.sync.dma_start(out=tokw, in_=tokw_d)
            tokx = data_pool.tile([128, NBLK * 512], dt.float16)
            tokx_b = bass.AP(
                tensor=tokx_d.tensor, offset=tokx_d.offset, ap=[[0, 128], tokx_d.ap[1]]
            )
            nc.sync.dma_start(out=tokx, in_=tokx_b)
            nc.sync.dma_start(out=wall, in_=wT_r)
            nc.sync.dma_start(out=xall[..., 0:512], in_=xT_r[..., 0:512])
            for b in range(1, NBLK):
                nc.sync.dma_start(
                    out=xall[..., b * 512 : (b + 1) * 512],
                    in_=xT_r[..., b * 512 : (b + 1) * 512],
                )
            n_ksteps = 4 if fp8 else KT

            zbias = data_pool.tile([128, 1], dt.float32)
            nc.vector.memset(zbias, 0.0)
            spacc = data_pool.tile([128, 2 * repeat], dt.float32)

            # PE warmup: the HAM clock gate needs ~3.4us of sustained PE
            # activity to unthrottle 1.2 -> 2.4 GHz.  Run garbage matmuls on
            # a memset tile while the first DMAs land so the real matmuls
            # start warm.
            dummy = data_pool.tile([128, 128], dt.bfloat16)
            nc.vector.memset(dummy, 0.0)
            dummy_ps = ps.tile([128, 512], dt.float32, name="dummy_ps", bufs=1)
            for _ in range(34):
                nc.tensor.matmul(dummy_ps[:, 0:128], dummy, dummy, start=True, stop=True)

            sgn_eng = nc.vector if sgn_engine == "vector" else nc.gpsimd
            PIPE = pipe  # delay product-reduces so they don't stall DVE's queue
            sgn_const = None
            if probe == "nosign":
                sgn_const = scratch.tile([128, 512], dt.float16, name="sgn_const", bufs=1)
                nc.vector.memset(sgn_const, -0.5)
            for r in range(repeat):
                pacc = scratch.tile([128, T * GC], dt.float16, name="pacc", bufs=2)
                s_tiles = {}

                def emit_reduce(t):
                    col = pacc_col[t]
                    nc.vector.tensor_reduce(
                        out=pacc[:, col : col + GC],
                        in_=s_tiles.pop(t).rearrange("a (g e) -> a g e", e=PG),
                        axis=mybir.AxisListType.X,
                        op=mybir.AluOpType.mult,
                    )

                pending = []

                def consume_tile(t, zp):
                    if probe == "pe":
                        return
                    w, x = t % 4, t // 4
                    # sign' = (tokx == tokw) - 0.5: +0.5 match, -0.5 not
                    if probe == "nosign":
                        sgn = sgn_const
                    else:
                        sgn = scratch.tile([128, 512], dt.float16, name="sgn")
                        sgn_eng.tensor_scalar(
                            out=sgn,
                            in0=tokx[:, x * 512 : (x + 1) * 512],
                            scalar1=tokw[:, w : w + 1],
                            scalar2=0.5,
                            op0=mybir.AluOpType.is_equal,
                            op1=mybir.AluOpType.subtract,
                        )
                    # v = z * sign'   (w := z*(1-2t) = -2v)
                    v = scratch.tile([128, 512], dt.float32, name="v")
                    nc.vector.tensor_tensor(
                        out=v, in0=zp, in1=sgn, op=mybir.AluOpType.mult
                    )
                    # s = sigmoid(2v) = sigma(-w);  softplus(w) = -ln(s)
                    s = scratch.tile([128, 512], dt.float16, name="s", bufs=PIPE + 2)
                    nc.scalar.activation(
                        out=s,
                        in_=v,
                        func=mybir.ActivationFunctionType.Sigmoid,
                        bias=zbias,
                        scale=2.0,
                    )
                    s_tiles[t] = s
                    pending.append(t)
                    # grouped products, software-pipelined PIPE tiles behind
                    # so the reduce never stalls DVE's in-order queue
                    if len(pending) > PIPE and probe != "noreduce":
                        emit_reduce(pending.pop(0))

                for t in range(T):
                    w, x = t % 4, t // 4
                    zp = ps.tile([128, 512], dt.float32, name="zp")
                    for k in range(n_ksteps):
                        if fp8:
                            nc.tensor.matmul(
                                zp,
                                wall[:, k, :, w * 128 : (w + 1) * 128],
                                xall[:, k, :, x * 512 : (x + 1) * 512],
                                start=(k == 0),
                                stop=(k == n_ksteps - 1),
                                perf_mode=mybir.MatmulPerfMode.DoubleRow,
                            )
                        else:
                            nc.tensor.matmul(
                                zp,
                                wall[:, k, w * 128 : (w + 1) * 128],
                                xall[:, k, x * 512 : (x + 1) * 512],
                                start=(k == 0),
                                stop=(k == n_ksteps - 1),
                            )
                    consume_tile(t, zp)

                if probe in ("pe", "noreduce"):
                    nc.vector.memset(pacc, 0.5)
                    s_tiles.clear()
                else:
                    for t in list(pending):
                        emit_reduce(t)
                # two tail ln+accum passes, one per host weight class
                junk1 = scratch.tile([128, NW1], dt.float32, name="junk1")
                nc.scalar.activation(
                    out=junk1,
                    in_=pacc[:, :NW1],
                    func=mybir.ActivationFunctionType.Ln,
                    bias=zbias,
                    scale=1.0,
                    accum_out=spacc[:, 2 * r : 2 * r + 1],
                )
                junk2 = scratch.tile([128, T * GC - NW1], dt.float32, name="junk2")
                nc.scalar.activation(
                    out=junk2,
                    in_=pacc[:, NW1:],
                    func=mybir.ActivationFunctionType.Ln,
                    bias=zbias,
                    scale=1.0,
                    accum_out=spacc[:, 2 * r + 1 : 2 * r + 2],
                )

            nc.sync.dma_start(out=sp_d, in_=spacc)

    nc.compile()
    return nc


def prep_in_maps(data, token_ids, indices):
    data = np.asarray(data, dtype=np.float32)
    token_ids = np.asarray(token_ids)
    indices = np.asarray(indices)

    # host prep: gather, normalize, transpose, quantize
    x = data[indices]  # [M, D] f32
    norms = np.sqrt((x.astype(np.float64) ** 2).sum(-1))
    xh = (x / np.maximum(norms[:, None], 1e-8)).astype(np.float32)
    # DoubleRow fp8 layout: X8[k', p, j, col] = xh[col, k'*256 + 2p + j]
    X8 = np.ascontiguousarray(
        xh.T.reshape(4, 128, 2, M).astype(ml_dtypes.float8_e4m3)
    )
    tok = token_ids[indices]  # tokx fp16 (0..511 exact), tokw f32 (scalar op requires f32)

    in_maps = []
    for c in range(N_CORES):
        groups = [(c + j) % N_CORES for j in range(NBLK)]
        x8 = np.concatenate(
            [X8[:, :, :, g * GROUP : (g + 1) * GROUP] for g in groups], axis=3
        )
        tokx = np.concatenate([tok[g * GROUP : (g + 1) * GROUP] for g in groups])
        in_maps.append(
            {
                "wT": np.ascontiguousarray(
                    X8[:, :, :, c * GROUP : (c + 1) * GROUP].transpose(1, 0, 2, 3)
                ).reshape(128, -1),
                "xT": np.ascontiguousarray(x8.transpose(1, 0, 2, 3)).reshape(128, -1),
                "tokx": np.ascontiguousarray(tokx.reshape(1, -1).astype(np.float16)),
                "tokw": np.ascontiguousarray(
                    tok[c * GROUP : (c + 1) * GROUP].reshape(4, 128).T.astype(np.float32)
                ),
            }
        )
    return in_maps


def kernel(data, token_ids, indices):
    global last_result
    in_maps = prep_in_maps(data, token_ids, indices)

    if "nc" not in _cache:
        _cache["nc"] = _build()
    nc = _cache["nc"]

    trace = os.environ.get("KERNEL_PROFILE", "") == "1"
    res = run_bass_kernel_spmd(nc, in_maps, list(range(N_CORES)), trace=trace)
    last_result = res

    total = 0.0
    for c in range(N_CORES):
        sp = res.results[c]["spacc"].astype(np.float64)  # [128, 2]
        total += sp[:, 0].sum() + 2.0 * sp[:, 1].sum()
    loss = -total / (M * M)  # spacc holds ln(sigma) sums = -softplus sums
    return np.float32(loss)



# revision 27
# speedup vs baseline: 10.4221x; 10.4221x over previous
"""Trainium2 Bass kernel v2 for nn_CharacterLoss: pairwise-cosine BCE loss.

reference:  x = data[indices]; z = cosine-sim(x, x)  [M, M]
            t = token match;  loss = mean(softplus(z) - z * t)

Pointwise identity used per entry: softplus(z) - z*t = softplus(z*(1-2t)),
and softplus(w) = -ln sigma(-w), so summing ln of sigmoids gives the loss.

v2 changes vs v1 (both math and schedule):
1. HOST SORTS THE GATHERED ROWS BY TOKEN (a permutation of the rows does
   not change the all-pairs loss).  After sorting, token matches (t=1)
   only occur between rows of the same contiguous segment, so t != 0 only
   within a narrow diagonal band of the pairwise matrix.  Off-band tiles
   skip the DVE sign/multiply entirely: ACT reads PSUM directly with
   s = sigma(-z).  Only the band subtiles (diag + BAND neighbors) run the
   v1 sign' = (tok==tok)-0.5, v = z*sign', s = sigma(2v) path.
2. UPPER-TRIANGLE SCHEDULE at [128,128] subtile granularity.  The 32x32
   subtile grid's unique work is wrap-diagonals i=0..16.  Core c owns row
   strips {c, c+8, c+16, c+24}; strip r computes wrap-cols r..r+W-1
   (W=17 for slots 0,1 / W=16 for slots 2,3 -- strips >= 16 skip diag 16,
   which their partner strip < 16 covers with weight 2).  66 subtiles per
   core vs v1's 80 (-17.5% PE work).  The program is SPMD-uniform: the
   per-core X operand is stored column-ROTATED (cols c, c+1, ... mod 32,
   with 9 wrap subtiles duplicated) so slot s always reads offset 8s.
3. WEIGHT-STATIONARY PE ORDER with ldweights=False on chunks that reuse
   the previous matmul's stationary tile (k-outer, column-chunk-inner).
Host-side weights for the final sum: wrap-diag 0 -> 1, 1..15 -> 2,
16 -> 2 (computed once, by the strip < 16 of each pair).

Perf (device-loop rig, For_i x 257 around 16 inline bodies, med-slope,
one machine session): v1 30580 ns/body, v2 20505, v4 (host band) 19073,
v5 (host band + host ln tail) 14742 = the PE-only probe floor (13932) --
the shipped config is PE-stream-bound.  Breakdown of the wins:
 - triangle schedule:      -17.5% PE stream cycles (v1 -> v2)
 - ldweights=False reuse:  ~ -4% (v2 vs v2nold)
 - BAND_MODE="host": the exact linear term sum z*t
   (= sum_g ||sum_{i in g} xq_i||^2, O(M*D) f64 on host, same
   quantized rows the device multiplies) removes all DVE band ops
   from the consume chain (-1.6us)
 - TAIL_MODE="host": second-level DVE product-reduce (products of 64
   sigmoids, >= 6e-23, f32-safe) ships [128,132] partials; host does
   the final ln + weighted sum in f64.  The body then touches only the
   sigmoid ACT table: no per-body table reloads (2 x 1283ns) and no Ln
   instructions (-4.3us).
HW rel err 4.3e-7 vs the fp32 reference (band/tail modes are exact
decompositions; fp8 quantization is the only approximation).
"""
import os
import sys

sys.path.insert(0, "/opt/trn_rl_repo")

import numpy as np
import ml_dtypes

import concourse.bass as bass
import concourse.mybir as mybir
import concourse.tile as tile
from concourse import bacc
from concourse.bass_utils import run_bass_kernel_spmd

N_CORES = 8
M = 4096
D = 1024
NSUB = M // 128  # 32 subtile strips
SLOTS = 4  # strips per core: c, c+8, c+16, c+24
WIDTHS = [17, 17, 16, 16]  # wrap-cols per slot (strip<16 covers diag 16)
XSUB = 41  # rotated X cols shipped: 32 + 9 wrap duplicates
PG = 8  # product-group size for the sigmoid reduce
_cache = {}
last_result = None


def _build(repeat=1, band=1, ldw_reuse=True, probe="", timing=False, ksteps=4, loopn=None, psab=(2, 1), band_mode="device", tail_mode="device"):
    """timing=True replaces the big DRAM input loads with on-device memsets
    (identical body instruction stream; matmul/DVE/ACT timing is
    data-independent) so benchmark calls don't ship MBs through the axon
    tunnel each call.  ksteps: DoubleRow k-steps (contraction = 256*ksteps).
    loopn: wrap the repeat bodies in a device-side For_i loop (timing rig:
    total bodies = loopn * repeat per execution)."""
    nc = bacc.Bacc("TRN2", target_bir_lowering=False, debug=False)
    dt = mybir.dt
    BC = 128 * (1 + band)  # band columns per slot (sign-trick region)
    if not timing:
        wT_d = nc.dram_tensor(
            "wT", [128, ksteps * 2 * SLOTS * 128], dt.float8e4, kind="ExternalInput"
        ).ap()
        xT_d = nc.dram_tensor(
            "xT", [128, ksteps * 2 * XSUB * 128], dt.float8e4, kind="ExternalInput"
        ).ap()
        tokx_d = nc.dram_tensor(
            "tokx", [1, SLOTS * BC], dt.float16, kind="ExternalInput"
        ).ap()
        tokw_d = nc.dram_tensor(
            "tokw", [128, SLOTS], dt.float32, kind="ExternalInput"
        ).ap()
    SPW = 2 if tail_mode == "device" else 132  # cols shipped back per body
    sp_d = nc.dram_tensor(
        "spacc", [128, SPW * repeat], dt.float32, kind="ExternalOutput"
    ).ap()

    # pacc layout per body: [w1: SLOTS*16 | w2: SLOTS*(W-1)*16] fp16 columns.
    # Slot s: diag subtile i=0 -> w1[16s:16s+16]; i=1..W-1 -> w2 block of
    # (W-1)*16 at W2OFF + s*240 (slots 2,3 use 15 subtiles = 240 of 256... use
    # per-slot stride 16*(17-1)=256 padded? keep exact: per-slot w2 width
    # depends on slot: 16*16=256 for slots 0,1 and 15*16=240 for 2,3).
    W1W = SLOTS * 16
    w2off = [0] * SLOTS
    off = W1W
    for s in range(SLOTS):
        w2off[s] = off
        off += (WIDTHS[s] - 1) * 16
    PACCW = off  # 64 + 256+256+240+240 = 1056

    with tile.TileContext(nc) as tc:
        with (
            tc.tile_pool(name="data", bufs=1) as data_pool,
            tc.tile_pool(name="scratch", bufs=3) as scratch,
            tc.tile_pool(name="ps", bufs=1, space="PSUM") as ps,
        ):
            wall = data_pool.tile([128, ksteps, 2, SLOTS * 128], dt.float8e4)
            xall = data_pool.tile([128, ksteps, 2, XSUB * 128], dt.float8e4)
            tokw = data_pool.tile([128, SLOTS], dt.float32)
            tokx = data_pool.tile([128, SLOTS * BC], dt.float16)
            if timing:
                nc.vector.memset(tokw, 1.0)
                nc.vector.memset(tokx, 1.0)
                nc.vector.memset(wall, 0.0)
                nc.vector.memset(xall, 0.0)
            else:
                wT_r = wT_d.rearrange("p (k j c) -> p k j c", k=ksteps, j=2)
                xT_r = xT_d.rearrange("p (k j c) -> p k j c", k=ksteps, j=2)
                nc.sync.dma_start(out=tokw, in_=tokw_d)
                tokx_b = bass.AP(
                    tensor=tokx_d.tensor,
                    offset=tokx_d.offset,
                    ap=[[0, 128], tokx_d.ap[1]],
                )
                nc.sync.dma_start(out=tokx, in_=tokx_b)
                nc.sync.dma_start(out=wall, in_=wT_r)
                # x in disjoint pieces whose prefix-union covers slot s's
                # range by piece s, so the first slot's matmuls aren't gated
                # on the whole 5.2MB transfer
                bounds = [0, 17, 25, 33, XSUB]
                for b in range(SLOTS):
                    lo, hi = bounds[b] * 128, bounds[b + 1] * 128
                    nc.sync.dma_start(out=xall[..., lo:hi], in_=xT_r[..., lo:hi])

            zbias = data_pool.tile([128, 1], dt.float32)
            nc.vector.memset(zbias, 0.0)
            spacc = data_pool.tile([128, SPW * repeat], dt.float32)

            # PE p-state warmup while DMAs land
            dummy = data_pool.tile([128, 128], dt.bfloat16)
            nc.vector.memset(dummy, 0.0)
            # warmup shares the psA ring (a bank stays free for psB bufs=2)
            dummy_ps = ps.tile([128, 1024], dt.float32, name="psA", bufs=psab[0])
            for _ in range(34):
                nc.tensor.matmul(dummy_ps[:, 0:128], dummy, dummy, start=True, stop=True)

            import contextlib

            loop_cm = tc.For_i(0, loopn) if loopn else contextlib.nullcontext()
            with loop_cm:
              for r in range(repeat):
                pacc = scratch.tile([128, PACCW], dt.float16, name="pacc", bufs=2)

                # hoist the sign tiles off the consume critical chain: they
                # depend only on tokens, so DVE computes all 4 while the PE
                # streams slot 0
                sgns = {}
                if probe != "pe" and band_mode == "device":
                    for s in range(SLOTS):
                        sgn = scratch.tile(
                            [128, BC], dt.float16, name="sgn", bufs=SLOTS + 1
                        )
                        nc.vector.tensor_scalar(
                            out=sgn,
                            in0=tokx[:, s * BC : (s + 1) * BC],
                            scalar1=tokw[:, s : s + 1],
                            scalar2=0.5,
                            op0=mybir.AluOpType.is_equal,
                            op1=mybir.AluOpType.subtract,
                        )
                        sgns[s] = sgn

                def consume_unit(s, zp, cols, sub0):
                    """sigmoids + product-reduce for subtiles sub0..sub0+cols/128
                    of slot s held in psum tile zp[:, :cols]."""
                    if probe == "pe":
                        return
                    segs = []  # (s-tile, col offset within unit) in stream order
                    c0 = 0
                    if sub0 == 0 and probe != "act" and band_mode == "device":
                        # band region: sign-trick (t can be nonzero here)
                        v = scratch.tile([128, BC], dt.float32, name="v")
                        nc.vector.tensor_tensor(
                            out=v, in0=zp[:, 0:BC], in1=sgns[s], op=mybir.AluOpType.mult
                        )
                        sb = scratch.tile([128, BC], dt.float16, name="sband")
                        nc.scalar.activation(
                            out=sb,
                            in_=v,
                            func=mybir.ActivationFunctionType.Sigmoid,
                            bias=zbias,
                            scale=2.0,
                        )
                        segs.append((sb, 0))
                        c0 = BC
                    if cols > c0:
                        sp = scratch.tile(
                            [128, cols - c0], dt.float16, name=f"spl{cols - c0}", bufs=2
                        )
                        nc.scalar.activation(
                            out=sp,
                            in_=zp[:, c0:cols],
                            func=mybir.ActivationFunctionType.Sigmoid,
                            bias=zbias,
                            scale=-1.0,
                        )
                        segs.append((sp, c0))
                    if probe == "act":
                        return
                    # product-reduce into pacc: subtile i -> w1 (i==0) else w2
                    for st, base in segs:
                        ncols = st.shape[-1]
                        # split the s-tile at subtile boundaries by weight class
                        pieces = []
                        i0 = (sub0 * 128 + base) // 128
                        n = ncols // 128
                        if i0 == 0:
                            pieces.append((0, 128, 16 * s))  # diag subtile -> w1
                            if n > 1:
                                pieces.append((128, ncols, w2off[s]))
                        else:
                            pieces.append((0, ncols, w2off[s] + (i0 - 1) * 16))
                        for lo, hi, tgt in pieces:
                            nc.vector.tensor_reduce(
                                out=pacc[:, tgt : tgt + (hi - lo) // PG],
                                in_=st[:, lo:hi].rearrange("a (g e) -> a g e", e=PG),
                                axis=mybir.AxisListType.X,
                                op=mybir.AluOpType.mult,
                            )

                for s in range(SLOTS):
                    W = WIDTHS[s]
                    xbase = 8 * s * 128
                    # unit A: subtiles 0..7 (1024 cols), unit B: 8..W-1
                    for sub0, nsub, pname, pcols in (
                        (0, 8, "psA", 1024),
                        (8, W - 8, "psB", 1152),
                    ):
                        cols = nsub * 128
                        zp = ps.tile(
                            [128, pcols],
                            dt.float32,
                            name=pname,
                            bufs=psab[0] if pname == "psA" else psab[1],
                        )
                        for k in range(ksteps):
                            prev_w = None
                            for clo in range(0, cols, 512):
                                chi = min(clo + 512, cols)
                                mm = nc.tensor.matmul(
                                    zp[:, clo:chi],
                                    wall[:, k, :, s * 128 : (s + 1) * 128],
                                    xall[:, k, :, xbase + sub0 * 128 + clo : xbase + sub0 * 128 + chi],
                                    start=(k == 0),
                                    stop=(k == ksteps - 1),
                                    perf_mode=mybir.MatmulPerfMode.DoubleRow,
                                )
                                if ldw_reuse and prev_w is not None:
                                    mm.ldweights = False
                                prev_w = k
                        consume_unit(s, zp, cols, sub0)

                if probe in ("pe", "act"):
                    nc.vector.memset(pacc, 0.5)
                if tail_mode == "host":
                    # second-level product reduce (products of 64 sigmoids,
                    # >= 6e-23, safe in f32); host does the final ln + sum.
                    # The body then only ever uses the sigmoid ACT table:
                    # no per-body table reloads, no Ln instructions.
                    nc.vector.tensor_reduce(
                        out=spacc[:, SPW * r : SPW * (r + 1)],
                        in_=pacc.rearrange("a (g e) -> a g e", e=PG),
                        axis=mybir.AxisListType.X,
                        op=mybir.AluOpType.mult,
                    )
                else:
                    # tail: ln+accumulate per host-weight class
                    junk1 = scratch.tile([128, W1W], dt.float32, name="junk1")
                    nc.scalar.activation(
                        out=junk1,
                        in_=pacc[:, :W1W],
                        func=mybir.ActivationFunctionType.Ln,
                        bias=zbias,
                        scale=1.0,
                        accum_out=spacc[:, SPW * r : SPW * r + 1],
                    )
                    junk2 = scratch.tile([128, PACCW - W1W], dt.float32, name="junk2")
                    nc.scalar.activation(
                        out=junk2,
                        in_=pacc[:, W1W:],
                        func=mybir.ActivationFunctionType.Ln,
                        bias=zbias,
                        scale=1.0,
                        accum_out=spacc[:, SPW * r + 1 : SPW * r + 2],
                    )

            nc.sync.dma_start(out=sp_d, in_=spacc)

    nc.compile()
    return nc


def _sorted_rows(data, token_ids, indices):
    data = np.asarray(data, dtype=np.float32)
    token_ids = np.asarray(token_ids)
    indices = np.asarray(indices)
    tok_g = token_ids[indices]
    perm = np.argsort(tok_g, kind="stable")
    tok = tok_g[perm]  # sorted tokens, [M]
    x = data[indices][perm]  # [M, D] rows sorted by token
    norms = np.sqrt((x.astype(np.float64) ** 2).sum(-1))
    xh = (x / np.maximum(norms[:, None], 1e-8)).astype(np.float32)
    return xh, tok


def pack_maps(xh, tok, ksteps=4):
    """xh: [M, 256*ksteps] normalized rows sorted by token."""
    # DoubleRow fp8 layout: X8[k', p, j, col] = xh[col, k'*256 + 2p + j]
    X8 = np.ascontiguousarray(
        xh.T.reshape(ksteps, 128, 2, M).astype(ml_dtypes.float8_e4m3)
    )

    # band width check: every same-token segment must stay within one
    # subtile-diagonal of its start (BAND=1), i.e. no segment crosses more
    # than one 128-row boundary
    starts = np.flatnonzero(np.r_[True, tok[1:] != tok[:-1]])
    ends = np.r_[starts[1:], M]
    band = int(np.max((ends - 1) // 128 - starts // 128)) if len(starts) else 0
    band = max(band, 1)
    assert band <= 2, f"token segment too long for band schedule (band={band})"
    global BAND_USED, CORR
    BAND_USED = band
    # exact linear band term for band_mode="host":
    # sum_ij z_ij t_ij = sum_g ||sum_{i in g} xq_i||^2 over token groups,
    # computed from the same quantized rows the device multiplies
    xq = xh.astype(ml_dtypes.float8_e4m3).astype(np.float64)
    gsums = np.add.reduceat(xq, starts, axis=0)
    CORR = float((gsums**2).sum())

    BC = 128 * (1 + band)
    in_maps = []
    for c in range(N_CORES):
        strips = [(c + 8 * s) % NSUB for s in range(SLOTS)]
        rot = (c + np.arange(XSUB)) % NSUB  # rotated col subtiles, wrap dup
        cols = (rot[:, None] * 128 + np.arange(128)).ravel()
        x8 = X8[:, :, :, cols]  # [4, 128, 2, XSUB*128]
        w8 = np.concatenate(
            [X8[:, :, :, r * 128 : (r + 1) * 128] for r in strips], axis=3
        )
        tokx = np.concatenate(
            [tok[(np.arange(BC) + r * 128) % M] for r in strips]
        )  # band tokens per slot (wrap index harmless: only reached off-band)
        tokw = np.stack([tok[r * 128 : (r + 1) * 128] for r in strips], axis=1)
        in_maps.append(
            {
                "wT": np.ascontiguousarray(w8.transpose(1, 0, 2, 3)).reshape(128, -1),
                "xT": np.ascontiguousarray(x8.transpose(1, 0, 2, 3)).reshape(128, -1),
                "tokx": np.ascontiguousarray(tokx.reshape(1, -1).astype(np.float16)),
                "tokw": np.ascontiguousarray(tokw.astype(np.float32)),
            }
        )
    return in_maps


def prep_in_maps(data, token_ids, indices):
    xh, tok = _sorted_rows(data, token_ids, indices)
    return pack_maps(xh, tok, ksteps=4)


BAND_USED = 1
CORR = 0.0
BAND_MODE = "host"
TAIL_MODE = "host"


def kernel(data, token_ids, indices):
    global last_result
    in_maps = prep_in_maps(data, token_ids, indices)
    band = BAND_USED

    key = ("nc", band, BAND_MODE, TAIL_MODE)
    if key not in _cache:
        _cache[key] = _build(band=band, band_mode=BAND_MODE, tail_mode=TAIL_MODE)
    nc = _cache[key]

    trace = os.environ.get("KERNEL_PROFILE", "") == "1"
    res = run_bass_kernel_spmd(nc, in_maps, list(range(N_CORES)), trace=trace)
    last_result = res

    total = 0.0
    for c in range(N_CORES):
        sp = res.results[c]["spacc"].astype(np.float64)
        if TAIL_MODE == "host":
            # sp: [128, 132] products-of-64 sigmoids; w1 = first 8 cols
            lns = np.log(sp)
            total += lns[:, :8].sum() + 2.0 * lns[:, 8:].sum()
        else:
            total += sp[:, 0].sum() + 2.0 * sp[:, 1].sum()
    # spacc holds ln(sigma) sums = -softplus sums; in host band mode the
    # device summed plain softplus(z) and the exact -sum z*t is added here
    corr = CORR if BAND_MODE == "host" else 0.0
    loss = (-total - corr) / (M * M)
    return np.float32(loss)
